# revision 1
# baseline (speedup 1.0000x reference)
"""nn_MultiHeadPAttention Trainium2 kernel: full inputs -> full output.

Two SPMD launches on 8 NeuronCores, sequence-parallel (256 rows/core):
  launch 1: pattention x3 + rmsnorm + rotary per slice
  host:     gather k/v, transpose q/k, ones-augment v
  launch 2: full-softmax attention + Wo per slice
"""
import numpy as np

import mhpa_kernels as K
import concourse.tile as tile
from concourse import bacc, mybir
from concourse.bass_utils import run_bass_kernel_spmd

_CACHE = {}


def _get_program(which):
    if which in _CACHE:
        return _CACHE[which]
    build, io = ((K.build_kernel1, K.kernel1_io) if which == 1
                 else (K.build_kernel2, K.kernel2_io))
    ins_spec, outs_spec = io()
    nc = bacc.Bacc("TRN2", target_bir_lowering=False, debug=False,
                   num_devices=K.NC)
    in_aps = {k: nc.dram_tensor(k, shp, d, kind="ExternalInput").ap()
              for k, (shp, d) in ins_spec.items()}
    out_aps = {k: nc.dram_tensor(k, shp, d, kind="ExternalOutput").ap()
               for k, (shp, d) in outs_spec.items()}
    with tile.TileContext(nc) as tc:
        build(nc, tc, in_aps, out_aps)
    nc.compile()
    _CACHE[which] = nc
    return nc


def run_phase(which, in_maps, trace=False, tmpdir=None):
    nc = _get_program(which)
    return run_bass_kernel_spmd(nc, in_maps, core_ids=list(range(K.NC)),
                                trace=trace, tmpdir=tmpdir)


def kernel(**inputs):
    res1 = run_phase(1, K.host_prep1(inputs))
    q_rot = np.concatenate([res1.results[c]["q_rot"] for c in range(K.NC)], 0)
    k_rot = np.concatenate([res1.results[c]["k_rot"] for c in range(K.NC)], 0)
    v_out = np.concatenate([res1.results[c]["v_out"] for c in range(K.NC)], 0)

    res2 = run_phase(2, K.host_prep2(q_rot, k_rot, v_out, inputs["Wo"]))
    out = np.concatenate([res2.results[c]["out"] for c in range(K.NC)], 0)
    return out.astype(np.float32)
